# revision 18
# baseline (speedup 1.0000x reference)
"""Trainium2 Bass kernel for nn_DeformationNetworkGcnHybrid (GNN message passing).

Network: MLP1 (3-512-512-256-256, LeakyReLU) -> GraphConv(259->256)+ReLU ->
9x GraphConv(256->256)+ReLU -> MLP3 (256-128-64-32-3, LeakyReLU on first 3).
GraphConv(x) = x@w0.T + b0 + scatter_add_over_edges(x@w1.T + b1).

Strategy (8 NeuronCores, SPMD):
 - Vertices (64000, padded to 65536) are degree-sorted and dealt round-robin
   to 8 cores (8192 each). All per-vertex matmuls are row-sharded, bf16 on
   the PE with fp32 PSUM accumulation, activations kept feature-major
   ("xT" [256, 8192] per core) so layer weights stay stationary.
 - Scatter-add: per layer, each core computes w1x for its verts (vert-major),
   AllGathers it into a 32 MB bf16 DRAM table (row order chosen so the
   assembly DMA is byte-linear), then gathers its ~48k incoming contribution
   rows with the GPSIMD dma_gather instruction (dst-major slot layout,
   window-uniform slot counts, int16 indices split by table half, padding
   via negative indices which the ucode zero-fills), and reduces slots with
   a vector-engine pairwise tree (bf16 inputs, fp32 partials).
 - agg returns to feature-major via an HBM bounce + hardware DMA-transpose
   (xbar), then x_{l+1} = relu(w0x + agg + b0) fused on the vector engine.

kernel(**inputs) takes the FULL problem inputs and returns the FULL output.
"""
import os
import sys

for _p in ("/opt/trn_rl_repo", "/root/.axon_site/_ro/trn_rl_repo"):
    if os.path.isdir(_p) and _p not in sys.path:
        sys.path.insert(0, _p)

import numpy as np
import ml_dtypes

import concourse.bacc as bacc
import concourse.bass as bass
import concourse.tile as tile
from concourse import mybir
from concourse.bass_utils import run_bass_kernel_spmd

BF16 = ml_dtypes.bfloat16

N = 64000
NP = 65536
E = 192000
NCORE = 8
P = 128
NG = 64            # windows per core (128 dsts each)
VPC = NP // NCORE  # 8192
HALF = NP // 2
H = 256
BIN_CAP = 16       # max d-units (=128 idxs) per gather; 2048 idxs is the
                   # ucode limit (3072 crashed the exec unit)


# ----------------------------------------------------------------------------
# Host-side preprocessing
# ----------------------------------------------------------------------------

def _build_prep(edges):
    """Degree-dealt permutation + gather structures (uniform across cores)."""
    edges = np.asarray(edges).astype(np.int64)
    dst = np.concatenate([edges[:, 0], edges[:, 1]])
    src = np.concatenate([edges[:, 1], edges[:, 0]])
    deg = np.bincount(dst, minlength=NP)

    # pass 1: assign cores by global degree rank (round-robin)
    order = np.argsort(-deg, kind="stable")
    core_of_old = np.empty(NP, np.int64)
    core_of_old[order] = np.arange(NP) % NCORE

    # src half = owning core of src >= 4 (since R = c*8192 + r)
    half_of_old = (core_of_old >= NCORE // 2).astype(np.int64)
    # per-dst (c0, c1): counts of contributions by src half
    c_h = np.zeros((NP, 2), np.int64)
    np.add.at(c_h, (dst, half_of_old[src]), 1)

    # pass 2: within each core, sort dsts by (c0 desc, c1 desc) -> (p, g)
    old2l = np.empty(NP, np.int64)
    for c in range(NCORE):
        mine = np.where(core_of_old == c)[0]
        assert len(mine) == VPC
        k = np.lexsort((-c_h[mine, 1], -c_h[mine, 0]))
        mine = mine[k]
        s = np.arange(VPC)
        l_local = (s % P) * NG + s // P
        old2l[mine] = c * VPC + l_local
    l2old = np.empty(NP, np.int64)
    l2old[old2l] = np.arange(NP)

    ll = np.arange(NP)
    R_of_label = (ll // VPC) * VPC + ((ll % VPC) % P) * NG + (ll % VPC) // P

    # every core's last-sorted vert (l_local=8191 -> R=c*8192+8191) must be
    # deg-0: cores zero that w1x row pre-AllGather, giving the zero pad rows
    # at R=32767 (half-0 pad) and R=65535 (half-1 pad).
    for c in range(NCORE):
        lab = c * VPC + (VPC - 1)
        assert deg[l2old[lab]] == 0, "per-core pad vert must be deg-0"

    dst_l = old2l[dst]
    src_R = R_of_label[old2l[src]]
    halves = (src_R >= HALF).astype(np.int64)

    ord2 = np.argsort(dst_l * 2 + halves, kind="stable")
    dst_s = dst_l[ord2]
    srcR_s = src_R[ord2]
    half_s = halves[ord2]
    key = dst_s * 2 + half_s
    cnt = np.bincount(key, minlength=2 * NP).reshape(NP, 2)
    starts = np.zeros(2 * NP + 1, np.int64)
    np.cumsum(cnt.reshape(-1), out=starts[1:])

    # uniform window slot counts d[h][g] = max over all cores and the window
    d_u = np.zeros((2, NG), np.int64)
    for h in range(2):
        cc = cnt[:, h].reshape(NCORE, P, NG)
        d_u[h] = cc.max(axis=(0, 1))
    d_u[0] = np.maximum(d_u[0], 1)  # every window written by half-0 tree

    # gather bins: consecutive windows packed to <= BIN_CAP d-units per
    # gather instruction (one flat idx list); uniform-d tree runs subdivide
    # each bin. Decouples gather-instruction count from run raggedness.
    # bin = (a, b, cap, runs); run = (ra, rb, d, u0) with u0 = slot-units
    # (x128 slots) from the bin start to the run start.
    bins = {0: [], 1: []}
    for h in range(2):
        cur = None
        for g in range(NG):
            d = int(d_u[h][g])
            if d == 0:
                continue
            if cur is None or cur[2] + d > BIN_CAP:
                if cur is not None:
                    bins[h].append(cur)
                cur = [g, g + 1, 0, []]
            runs = cur[3]
            if runs and runs[-1][2] == d and runs[-1][1] == g:
                runs[-1] = (runs[-1][0], g + 1, d, runs[-1][3])
            else:
                runs.append((g, g + 1, d, cur[2]))
            cur[2] += d
            cur[1] = g + 1
        if cur is not None:
            bins[h].append(cur)

    wbase = {h: np.zeros(NG + 1, np.int64) for h in range(2)}
    for h in range(2):
        np.cumsum(d_u[h] * P, out=wbase[h][1:])

    # per-core idx arrays (flat slot order), padding = -1 (ucode zero-fills)
    idx_flat = np.full((NCORE, 2, int((d_u.sum(axis=1) * P).max()) + 1), -1,
                       np.int64)
    tot = {h: int(d_u[h].sum()) * P for h in range(2)}
    idx_per = []
    for c in range(NCORE):
        per_h = []
        for h in range(2):
            # padding -> 32767: local row of the zeroed pad rows (R=32767 and
            # R=65535, both deg-0 verts). Trailing negative idxs would be left
            # stale by the ucode, so padding must be real zero-row gathers.
            vals = np.full(tot[h], HALF - 1, np.int64)
            for g in range(NG):
                base = wbase[h][g]
                for p in range(P):
                    lv = c * VPC + p * NG + g
                    n = cnt[lv, h]
                    if n == 0:
                        continue
                    s0 = starts[lv * 2 + h]
                    vals[base + np.arange(n) * P + p] = (
                        srcR_s[s0 : s0 + n] - h * HALF
                    )
            per_h.append(vals)
        idx_per.append(per_h)

    # wrap idx per piece: [128, G*d*8] int16 per piece, concatenated
    def wrap16(v):
        n = len(v)
        a = v.reshape(n // 16, 16).T.astype(np.int16)  # [16, n/16]
        return np.tile(a, (8, 1))

    bin_off = {0: [], 1: []}
    idx_wrapped = []
    for c in range(NCORE):
        cols = []
        off = 0
        for h in range(2):
            for (a, b, cap, runs) in bins[h]:
                nidx = cap * P
                v = idx_per[c][h][wbase[h][a] : wbase[h][a] + nidx]
                cols.append(wrap16(v))
                if c == 0:
                    bin_off[h].append(off)
                off += nidx // 16
        idx_wrapped.append(np.concatenate(cols, axis=1))
    idx_wrapped = np.stack(idx_wrapped)  # [NCORE, 128, IDXW]

    return dict(
        old2l=old2l, l2old=l2old, d_u=d_u, bins=bins,
        bin_off=bin_off, idx_wrapped=idx_wrapped,
        IDXW=idx_wrapped.shape[2],
    )


def _prep_weight(w):
    """torch-style [out, in] -> lhsT/rhs layout [in, out] bf16."""
    return np.ascontiguousarray(np.asarray(w).T).astype(BF16)


def _prep_bias_P(b):
    """bias [out] -> per-partition layout [128, ceil(out/128)] fp32."""
    out = np.asarray(b).astype(np.float32)
    n = len(out)
    w = max(1, (n + P - 1) // P)
    a = np.zeros((P, w), np.float32)
    for mo in range(w):
        seg = out[mo * P : (mo + 1) * P]
        a[: len(seg), mo] = seg
    return a


# ----------------------------------------------------------------------------
# Device program
# ----------------------------------------------------------------------------

def _fap(tile_obj, offset, dims):
    base = tile_obj[:]
    return bass.AP(base.tensor, offset, [list(base.ap[0])] + [list(d) for d in dims])


def _emit_tree(nc, pool_g32, g16, base_u, G, d, agg_t, a, accumulate):
    """Reduce g16 slots [base_u*128 ... +G*d*128) ([G, d, 256] per partition)
    over d; write (or +=) agg_t[:, a*256...]."""
    F = H
    agg_ap = _fap(agg_t, a * F, [[F, G], [1, F]])
    cur, cur_d, off = g16, d, base_u * F
    while True:
        cur_step = cur_d * F
        if cur_d == 1:
            in0 = _fap(cur, off, [[cur_step, G], [1, F]])
            if accumulate:
                nc.vector.tensor_tensor(out=agg_ap, in0=agg_ap, in1=in0,
                                        op=mybir.AluOpType.add)
            else:
                nc.vector.tensor_copy(out=agg_ap, in_=in0)
            return
        if cur_d == 2 and not accumulate:
            in0 = _fap(cur, off, [[cur_step, G], [1, F]])
            in1 = _fap(cur, off + F, [[cur_step, G], [1, F]])
            nc.vector.tensor_tensor(out=agg_ap, in0=in0, in1=in1,
                                    op=mybir.AluOpType.add)
            return
        h, odd = cur_d // 2, cur_d % 2
        nxt = pool_g32.tile([P, G * (h + odd) * F], mybir.dt.float32, tag="g32")
        in0 = _fap(cur, off, [[cur_step, G], [2 * F, h], [1, F]])
        in1 = _fap(cur, off + F, [[cur_step, G], [2 * F, h], [1, F]])
        out = _fap(nxt, 0, [[(h + odd) * F, G], [F, h], [1, F]])
        nc.vector.tensor_tensor(out=out, in0=in0, in1=in1, op=mybir.AluOpType.add)
        if odd:
            in_l = _fap(cur, off + (cur_d - 1) * F, [[cur_step, G], [1, F]])
            out_l = _fap(nxt, h * F, [[(h + odd) * F, G], [1, F]])
            nc.vector.tensor_copy(out=out_l, in_=in_l)
        cur, cur_d, off = nxt, h + odd, 0


def build_program(pp, nreps=1):
    prep = pp
    IDXW = prep["IDXW"]
    nc = bacc.Bacc("TRN2", target_bir_lowering=False, debug=False,
                   num_devices=NCORE)

    # ---- inputs
    vertsT_d = nc.dram_tensor("vertsT", [3, VPC], mybir.dt.float32,
                              kind="ExternalInput")
    gidx_d = nc.dram_tensor("gidx", [P, IDXW], mybir.dt.int16,
                            kind="ExternalInput")
    win = {}
    def w_in(name, shape, dt=mybir.dt.bfloat16):
        win[name] = nc.dram_tensor(name, list(shape), dt, kind="ExternalInput")
        return win[name]

    w_in("m1w0T", (3, 512)); w_in("m1w1T", (512, 512))
    w_in("m1w2T", (512, 256)); w_in("m1w3T", (256, 256))
    w_in("m1b0P", (P, 4), mybir.dt.float32); w_in("m1b1P", (P, 4), mybir.dt.float32)
    w_in("m1b2P", (P, 2), mybir.dt.float32); w_in("m1b3P", (P, 2), mybir.dt.float32)
    w_in("g0w0T", (259, 256)); w_in("g0w1T", (259, 256))
    w_in("gw0T", (9, 256, 256)); w_in("gw1T", (9, 256, 256))
    w_in("gb0P", (10, P, 2), mybir.dt.float32)
    w_in("gb1F", (10, P, 256), mybir.dt.float32)
    w_in("m3w0T", (256, 128)); w_in("m3w1T", (128, 64))
    w_in("m3w2T", (64, 32)); w_in("m3w3T", (32, 3))
    w_in("m3b0P", (P, 1), mybir.dt.float32); w_in("m3b1P", (64, 1), mybir.dt.float32)
    w_in("m3b2P", (32, 1), mybir.dt.float32); w_in("m3b3P", (3, 1), mybir.dt.float32)

    y_d = nc.dram_tensor("y", [3, VPC], mybir.dt.float32, kind="ExternalOutput")

    LRELU = mybir.ActivationFunctionType.Lrelu
    ADD = mybir.AluOpType.add
    MAXOP = mybir.AluOpType.max
    BF = mybir.dt.bfloat16
    F32 = mybir.dt.float32

    with tile.TileContext(nc) as tc:
        with (
            tc.tile_pool(name="dram", bufs=1, space="DRAM") as dram,
            tc.tile_pool(name="big", bufs=1) as pool_big,
            tc.tile_pool(name="gat", bufs=2) as pool_g16,
            tc.tile_pool(name="g32", bufs=2) as pool_g32,
            tc.tile_pool(name="wts", bufs=2) as pool_w,
            tc.tile_pool(name="cst", bufs=1) as pool_c,
            tc.tile_pool(name="tmp", bufs=2) as pool_t,
            tc.tile_pool(name="ps", bufs=6, space="PSUM") as pool_ps,
        ):
            ag_in = dram.tile([VPC, H], BF)
            aggD = dram.tile([VPC, H], BF)

            # ---- constant loads
            idx_t = pool_c.tile([P, IDXW], mybir.dt.int16)
            nc.sync.dma_start(idx_t[:], gidx_d[:])
            vT = pool_c.tile([3, VPC], BF)
            nc.gpsimd.dma_start(vT[:], vertsT_d[:])  # fp32 -> bf16 cast
            zrow = pool_c.tile([1, H], BF)
            nc.vector.memset(zrow[:], 0)

            def load_w(name, shape, dt=BF):
                t = pool_c.tile(list(shape), dt, tag=name)
                nc.sync.dma_start(t[:], win[name][:])
                return t

            m1w0 = load_w("m1w0T", (3, 512))

            # load weights with K on partitions: [K, M], K%128==0 -> [128, K//128, M]
            def load_wK(name, K, M):
                kk = K // P
                t = pool_c.tile([P, kk, M], BF, tag=name)
                nc.sync.dma_start(
                    t[:], win[name][:].rearrange("(kk p) m -> p kk m", p=P)
                )
                return t

            m1w1_t = load_wK("m1w1T", 512, 512)
            m1w2_t = load_wK("m1w2T", 512, 256)
            m1w3_t = load_wK("m1w3T", 256, 256)
            m1b = [load_w(f"m1b{i}P", ((P, 4) if i < 2 else (P, 2)), F32)
                   for i in range(4)]
            m3w0_t = load_wK("m3w0T", 256, 128)
            m3w1_t = load_w("m3w1T", (128, 64))
            m3w2_t = load_w("m3w2T", (64, 32))
            m3w3_t = load_w("m3w3T", (32, 3))
            m3b = [load_w("m3b0P", (P, 1), F32), load_w("m3b1P", (64, 1), F32),
                   load_w("m3b2P", (32, 1), F32), load_w("m3b3P", (3, 1), F32)]

            xT = pool_big.tile([P, 2 * VPC], BF, tag="xT")

            # ---- MLP1: verts -> xT, blocks of 2048
            for vb in range(4):
                vof = vb * 2048
                h1 = pool_big.tile([P, 4 * 2048], BF, tag="b32a")
                for mo in range(4):
                    for vc in range(4):
                        ps = pool_ps.tile([P, 512], F32, tag="ps")
                        nc.tensor.matmul(
                            ps[:], m1w0[:, mo * P : (mo + 1) * P],
                            vT[:, vof + vc * 512 : vof + (vc + 1) * 512],
                            start=True, stop=True,
                        )
                        nc.scalar.activation(
                            h1[:, mo * 2048 + vc * 512 : mo * 2048 + (vc + 1) * 512],
                            ps[:], LRELU, bias=m1b[0][:, mo : mo + 1], alpha=0.01,
                        )
                h2 = pool_big.tile([P, 4 * 2048], BF, tag="b32b")
                for mo in range(4):
                    for vc in range(4):
                        ps = pool_ps.tile([P, 512], F32, tag="ps")
                        for ki in range(4):
                            nc.tensor.matmul(
                                ps[:],
                                m1w1_t[:, ki, mo * P : (mo + 1) * P],
                                h1[:, ki * 2048 + vc * 512 : ki * 2048 + (vc + 1) * 512],
                                start=(ki == 0), stop=(ki == 3),
                            )
                        nc.scalar.activation(
                            h2[:, mo * 2048 + vc * 512 : mo * 2048 + (vc + 1) * 512],
                            ps[:], LRELU, bias=m1b[1][:, mo : mo + 1], alpha=0.01,
                        )
                h3 = pool_big.tile([P, 2 * 2048], BF, tag="b32a")
                for mo in range(2):
                    for vc in range(4):
                        ps = pool_ps.tile([P, 512], F32, tag="ps")
                        for ki in range(4):
                            nc.tensor.matmul(
                                ps[:],
                                m1w2_t[:, ki, mo * P : (mo + 1) * P],
                                h2[:, ki * 2048 + vc * 512 : ki * 2048 + (vc + 1) * 512],
                                start=(ki == 0), stop=(ki == 3),
                            )
                        nc.scalar.activation(
                            h3[:, mo * 2048 + vc * 512 : mo * 2048 + (vc + 1) * 512],
                            ps[:], LRELU, bias=m1b[2][:, mo : mo + 1], alpha=0.01,
                        )
                for mo in range(2):
                    for vc in range(4):
                        ps = pool_ps.tile([P, 512], F32, tag="ps")
                        for ki in range(2):
                            nc.tensor.matmul(
                                ps[:],
                                m1w3_t[:, ki, mo * P : (mo + 1) * P],
                                h3[:, ki * 2048 + vc * 512 : ki * 2048 + (vc + 1) * 512],
                                start=(ki == 0), stop=(ki == 1),
                            )
                        nc.scalar.activation(
                            xT[:, mo * VPC + vof + vc * 512 : mo * VPC + vof + (vc + 1) * 512],
                            ps[:], LRELU, bias=m1b[3][:, mo : mo + 1], alpha=0.01,
                        )

            # ---- GraphConv layers
            for rep in range(nreps):
              for l in range(10):
                if l == 0:
                    w0s = pool_w.tile([P, 2, H], BF, tag="w0")
                    nc.sync.dma_start(
                        w0s[:], win["g0w0T"][:256].rearrange("(kk p) m -> p kk m", p=P))
                    w1s = pool_w.tile([P, 2, H], BF, tag="w1")
                    nc.sync.dma_start(
                        w1s[:], win["g0w1T"][:256].rearrange("(kk p) m -> p kk m", p=P))
                    w0v = pool_w.tile([3, H], BF, tag="w0v")
                    nc.sync.dma_start(w0v[:], win["g0w0T"][256:259])
                    w1v = pool_w.tile([3, H], BF, tag="w1v")
                    nc.sync.dma_start(w1v[:], win["g0w1T"][256:259])
                else:
                    w0s = pool_w.tile([P, 2, H], BF, tag="w0")
                    nc.sync.dma_start(
                        w0s[:], win["gw0T"][l - 1].rearrange("(kk p) m -> p kk m", p=P))
                    w1s = pool_w.tile([P, 2, H], BF, tag="w1")
                    nc.sync.dma_start(
                        w1s[:], win["gw1T"][l - 1].rearrange("(kk p) m -> p kk m", p=P))
                b0 = pool_w.tile([P, 2], F32, tag="b0")
                nc.sync.dma_start(b0[:], win["gb0P"][l])
                b1 = pool_w.tile([P, H], F32, tag="b1")
                nc.sync.dma_start(b1[:], win["gb1F"][l])

                # w1x vert-major -> w1x16 [128, 64, 256]
                w1x16 = pool_big.tile([P, NG * H], BF, tag="b32a")
                for t in range(NG):
                    ps = pool_ps.tile([P, 256], F32, tag="ps")
                    for ki in range(2):
                        nc.tensor.matmul(
                            ps[:],
                            xT[:, ki * VPC + t * P : ki * VPC + (t + 1) * P],
                            w1s[:, ki, :],
                            start=(ki == 0), stop=(ki == 1 and l != 0),
                        )
                    if l == 0:
                        nc.tensor.matmul(
                            ps[:], vT[:, t * P : (t + 1) * P], w1v[:],
                            start=False, stop=True,
                        )
                    nc.vector.tensor_tensor(
                        out=w1x16[:, t * H : (t + 1) * H],
                        in0=ps[:], in1=b1[:],
                        op=ADD,
                    )

                # assembly (byte-linear), then zero the pad vert's row
                # (l_local=8191 -> ag_in row 8191) so R=c*8192+8191 are zero
                nc.sync.dma_start(ag_in[:], w1x16[:])
                nc.sync.dma_start(ag_in[VPC - 1 : VPC, :], zrow[:])
                table = dram.tile([NP, H], BF, addr_space="Shared", tag="table")
                nc.gpsimd.collective_compute(
                    "AllGather", mybir.AluOpType.bypass,
                    ins=[ag_in.opt()], outs=[table.opt()],
                    replica_groups=[list(range(NCORE))],
                )

                # gather (binned) + tree -> agg [128, 64, 256] bf16
                agg_t = pool_big.tile([P, NG * H], BF, tag="b32b")
                for hh in range(2):
                    for bi, (a, b, cap, runs) in enumerate(prep["bins"][hh]):
                        nidx = cap * P
                        off = prep["bin_off"][hh][bi]
                        g16 = pool_g16.tile([P, cap * H], BF, tag="g16")
                        nc.gpsimd.dma_gather(
                            out_ap=_fap(g16, 0, [[H, cap], [1, H]]),
                            in_ap=table[hh * HALF : (hh + 1) * HALF, :],
                            idxs_ap=idx_t[:, off : off + nidx // 16],
                            num_idxs=nidx,
                            num_idxs_reg=nidx,
                            elem_size=H,
                            single_packet=False,
                        )
                        for (ra, rb, dd, u0) in runs:
                            aa, uu = ra, u0
                            while aa < rb:  # G<=8 per tree call (g32 SBUF cap)
                                bb = min(aa + 8, rb)
                                _emit_tree(nc, pool_g32, g16, uu, bb - aa, dd,
                                           agg_t, aa, accumulate=(hh == 1))
                                uu += (bb - aa) * dd
                                aa = bb

                # agg -> DRAM (l-order rows) -> xbar transpose -> aggT
                nc.sync.dma_start(aggD[:], agg_t[:])
                aggT = pool_big.tile([P, 2 * VPC], BF, tag="b32a")
                for fh in range(2):
                    nc.sync.dma_start_transpose(
                        aggT[:, fh * VPC : (fh + 1) * VPC],
                        aggD[:, fh * P : (fh + 1) * P],
                    )

                # combine: xT = relu(w0x + agg + b0)  (in-place)
                for vc in range(16):
                    pss = []
                    for mo in range(2):
                        ps = pool_ps.tile([P, 512], F32, tag="ps")
                        for ki in range(2):
                            nc.tensor.matmul(
                                ps[:],
                                w0s[:, ki, mo * P : (mo + 1) * P],
                                xT[:, ki * VPC + vc * 512 : ki * VPC + (vc + 1) * 512],
                                start=(ki == 0), stop=(ki == 1 and l != 0),
                            )
                        if l == 0:
                            nc.tensor.matmul(
                                ps[:], w0v[:, mo * P : (mo + 1) * P],
                                vT[:, vc * 512 : (vc + 1) * 512],
                                start=False, stop=True,
                            )
                        pss.append(ps)
                    for mo in range(2):
                        t1 = pool_t.tile([P, 512], F32, tag="t1")
                        nc.vector.tensor_tensor(
                            out=t1[:], in0=pss[mo][:],
                            in1=aggT[:, mo * VPC + vc * 512 : mo * VPC + (vc + 1) * 512],
                            op=ADD,
                        )
                        nc.vector.tensor_scalar(
                            out=xT[:, mo * VPC + vc * 512 : mo * VPC + (vc + 1) * 512],
                            in0=t1[:],
                            scalar1=b0[:, mo : mo + 1],
                            scalar2=0.0,
                            op0=ADD, op1=MAXOP,
                        )

            # ---- MLP3
            for vc in range(16):
                sl = slice(vc * 512, (vc + 1) * 512)
                ps5 = pool_ps.tile([P, 512], F32, tag="ps")
                for ki in range(2):
                    nc.tensor.matmul(
                        ps5[:], m3w0_t[:, ki, :],
                        xT[:, ki * VPC + vc * 512 : ki * VPC + (vc + 1) * 512],
                        start=(ki == 0), stop=(ki == 1),
                    )
                h5 = pool_t.tile([P, 512], BF, tag="h5")
                nc.scalar.activation(h5[:], ps5[:], LRELU,
                                     bias=m3b[0][:, :1], alpha=0.01)
                ps6 = pool_ps.tile([64, 512], F32, tag="ps")
                nc.tensor.matmul(ps6[:], m3w1_t[:], h5[:], start=True, stop=True)
                h6 = pool_t.tile([64, 512], BF, tag="h6")
                nc.scalar.activation(h6[:], ps6[:], LRELU,
                                     bias=m3b[1][:, :1], alpha=0.01)
                ps7 = pool_ps.tile([32, 512], F32, tag="ps")
                nc.tensor.matmul(ps7[:], m3w2_t[:], h6[:], start=True, stop=True)
                h7 = pool_t.tile([32, 512], BF, tag="h7")
                nc.scalar.activation(h7[:], ps7[:], LRELU,
                                     bias=m3b[2][:, :1], alpha=0.01)
                ps8 = pool_ps.tile([3, 512], F32, tag="ps")
                nc.tensor.matmul(ps8[:], m3w3_t[:], h7[:], start=True, stop=True)
                yt = pool_t.tile([3, 512], F32, tag="t1")
                nc.vector.tensor_scalar(
                    out=yt[:], in0=ps8[:],
                    scalar1=m3b[3][:, :1], scalar2=None, op0=ADD,
                )
                nc.sync.dma_start(y_d[:, sl], yt[:])

    nc.compile()
    return nc


# ----------------------------------------------------------------------------
# Entry point
# ----------------------------------------------------------------------------

_CACHE = {}


def _build_inputs(pp, inputs):
    l2old = pp["l2old"]
    verts = np.asarray(inputs["verts"], np.float32).reshape(-1, 3)
    verts_pad = np.zeros((NP, 3), np.float32)
    verts_pad[:N] = verts
    verts_l = verts_pad[l2old]  # label order

    base = {
        "m1w0T": _prep_weight(inputs["m1_w0"]),
        "m1w1T": _prep_weight(inputs["m1_w1"]),
        "m1w2T": _prep_weight(inputs["m1_w2"]),
        "m1w3T": _prep_weight(inputs["m1_w3"]),
        "m3w0T": _prep_weight(inputs["m3_w0"]),
        "m3w1T": _prep_weight(inputs["m3_w1"]),
        "m3w2T": _prep_weight(inputs["m3_w2"]),
        "m3w3T": _prep_weight(inputs["m3_w3"]),
        "g0w0T": _prep_weight(inputs["g0_w0"]),
        "g0w1T": _prep_weight(inputs["g0_w1"]),
        "gw0T": np.stack([_prep_weight(inputs["g_w0"][i]) for i in range(9)]),
        "gw1T": np.stack([_prep_weight(inputs["g_w1"][i]) for i in range(9)]),
        "gb0P": np.stack(
            [_prep_bias_P(inputs["g0_b0"])]
            + [_prep_bias_P(inputs["g_b0"][i]) for i in range(9)]
        ),
        "gb1F": np.stack(
            [np.tile(np.asarray(inputs["g0_b1"], np.float32)[None, :], (P, 1))]
            + [np.tile(np.asarray(inputs["g_b1"][i], np.float32)[None, :], (P, 1))
               for i in range(9)]
        ),
        "m1b0P": _prep_bias_P(inputs["m1_b0"]),
        "m1b1P": _prep_bias_P(inputs["m1_b1"]),
        "m1b2P": _prep_bias_P(inputs["m1_b2"]),
        "m1b3P": _prep_bias_P(inputs["m1_b3"]),
        "m3b0P": _prep_bias_P(inputs["m3_b0"]),
        "m3b1P": _prep_bias_P(inputs["m3_b1"])[:64],
        "m3b2P": _prep_bias_P(inputs["m3_b2"])[:32],
        "m3b3P": _prep_bias_P(inputs["m3_b3"])[:3],
    }

    in_maps = []
    for c in range(NCORE):
        m = dict(base)
        m["vertsT"] = np.ascontiguousarray(
            verts_l[c * VPC : (c + 1) * VPC].T
        ).astype(np.float32)
        m["gidx"] = pp["idx_wrapped"][c]
        in_maps.append(m)
    return in_maps


def _assemble_output(pp, results):
    y_l = np.zeros((NP, 3), np.float32)
    for c in range(NCORE):
        y_l[c * VPC : (c + 1) * VPC] = results[c]["y"].T
    out = y_l[pp["old2l"][:N]]
    # wait: row old v lives at label old2l[v]
    return np.ascontiguousarray(out)


def kernel(**inputs):
    edges = np.asarray(inputs["edges"])
    key = ("k", edges.shape[0])
    pp = _build_prep(edges)
    nc = build_program(pp)
    in_maps = _build_inputs(pp, inputs)
    res = run_bass_kernel_spmd(nc, in_maps, core_ids=list(range(NCORE)))
    return _assemble_output(pp, res.results)

